# revision 11
# baseline (speedup 1.0000x reference)
"""Trainium2 Bass kernel for nn_Entropy (KDE local-entropy via histogram binning).

Contract: kernel(**inputs) takes the FULL input x (2,2,1,80,80) fp32 and
returns the FULL output (2,2,80,80) fp32, sharding internally across 8
NeuronCores (core = batch*2 + row-half of the 74x74 patch grid).

v2 layout (column-major): stage A runs transposed (image columns on
partitions), the division image flattens to pixel order j = c*43 + r.
The one-hot lives in flat [128, 2*3440] geometry (both 128-bin halves
side by side); the 7x7 box-sum is two shift trees over the flat axis
(vertical = shift +1, horizontal = shift +43), garbage-tolerant at
column/half boundaries (garbage lands in rows r>36 which are never
read).  The last horizontal level for c>=14 is offloaded to the PE as a
4-way identity-matmul accumulation (h = t1 + t1>>86 + t1>>172 + v7>>258)
with ACT copying PSUM back to fp16.  Stage C: G = K @ h via four 128x128
fp16 matmuls per 512-patch chunk, Ln on ACT, h*log muls on DVE, and a
256->1 reduction via a (-1/49)-vector matmul whose result is DMA'd
straight from PSUM.
"""
import os
import sys

import numpy as np

for _p in ("/opt/trn_rl_repo", "/root/.axon_site/_ro/trn_rl_repo"):
    if os.path.isdir(_p) and _p not in sys.path:
        sys.path.insert(0, _p)

import concourse.bass as bass
import concourse.bacc as bacc
import concourse.tile as tile
from concourse import mybir
from concourse.bass_utils import run_bass_kernel_spmd

dt = mybir.dt
Alu = mybir.AluOpType
Act = mybir.ActivationFunctionType
f32 = np.float32

R = 7
BW = 2.5
L = R * R  # 49
NORM = f32((2.0 * np.pi * BW * BW) ** 0.5)  # C=1 -> exponent 1/2
LN_SCALE = float(f32(1.0 / (L * NORM)))
INV25 = float(f32(1.0) / f32(25.0))
NEG_INV_L = float(-(f32(1.0) / f32(L)))
MAGIC = 8388608.0  # RNE(v) == (v + 2^23) - 2^23 for 0 <= v < 2^23

# geometry (per core: one 47x80 input strip -> 37x74 entropy block)
HP = 74          # patch grid cols (80 - 7 + 1)
ROWS = 43        # division-image rows per core (37 patch rows + 6)
PR = 37          # patch rows per core
NPIX = ROWS * 80         # 3440 pixels, flat j = c*43 + r (within a half)
NF = 2 * NPIX            # 6880: both halves
NP_ = PR * HP            # 2738 patches per core, flat p = c*37 + r'
CHUNK = 512              # stage C patch chunk
CSPLIT = 14              # hh: DVE computes c<14, PE c>=14 (5 chunks of 12)

_COMPILED = None


def _host_constants():
    f16 = np.float16
    bins = np.arange(256, dtype=np.float64)
    kmat = np.exp(-((bins[:, None] - bins[None, :]) ** 2) / (2.0 * BW * BW)).astype(f16)
    bh = np.zeros((84, 80), f32)
    for c in range(80):
        bh[c : c + 5, c] = 1.0
    binsA = np.arange(0, 128, dtype=f32).reshape(128, 1)
    binsB = np.arange(128, 256, dtype=f32).reshape(128, 1)
    # reduce weights carry the -1/49 entropy scale
    negones = np.full((128, 1), f16(NEG_INV_L), f16)
    onesrow = np.ones((1, 128), f16)
    ident = np.eye(128, dtype=f16)
    return {
        "kmat": kmat, "bh": bh, "binsA": binsA, "binsB": binsB,
        "ones": negones, "onesrow": onesrow, "ident": ident,
    }


def _build_nc():
    nc = bacc.Bacc("TRN2", target_bir_lowering=False, debug=False)

    xsT_d = nc.dram_tensor("xsT", [84, 47], dt.float32, kind="ExternalInput")
    xmT_d = nc.dram_tensor("xmT", [80, ROWS], dt.float32, kind="ExternalInput")
    bh_d = nc.dram_tensor("bh", [84, 80], dt.float32, kind="ExternalInput")
    kmat_d = nc.dram_tensor("kmat", [256, 256], dt.float16, kind="ExternalInput")
    binsA_d = nc.dram_tensor("binsA", [128, 1], dt.float32, kind="ExternalInput")
    binsB_d = nc.dram_tensor("binsB", [128, 1], dt.float32, kind="ExternalInput")
    ones_d = nc.dram_tensor("ones", [128, 1], dt.float16, kind="ExternalInput")
    onesrow_d = nc.dram_tensor("onesrow", [1, 128], dt.float16, kind="ExternalInput")
    ident_d = nc.dram_tensor("ident", [128, 128], dt.float16, kind="ExternalInput")
    ent_d = nc.dram_tensor("ent", [NP_], dt.float32, kind="ExternalOutput")

    cchunks = []
    off = 0
    while off < NP_:
        cw = min(CHUNK, NP_ - off)
        cchunks.append((off, cw))
        off += cw

    with tile.TileContext(nc) as tc:
        with (
            tc.tile_pool(name="small", bufs=1) as small,
            tc.tile_pool(name="pre", bufs=1) as pre,
            tc.tile_pool(name="big", bufs=1) as big,
            tc.tile_pool(name="scratch", bufs=1) as scratch,
            tc.tile_pool(name="psum", bufs=2, space="PSUM") as psum,
            tc.tile_pool(name="psum1", bufs=2, space="PSUM") as psum1,
        ):
            # ---------- input DMAs + constants ----------
            xsT = pre.tile([84, 47], dt.float32)
            nc.sync.dma_start(xsT[:], xsT_d[:])
            xmT = pre.tile([80, ROWS], dt.float32)
            nc.sync.dma_start(xmT[:], xmT_d[:])
            bht = pre.tile([84, 80], dt.float32)
            nc.sync.dma_start(bht[:], bh_d[:])

            eps_t = small.tile([128, 1], dt.float32)
            nc.vector.memset(eps_t[:], 1e-8)
            # preload both ACT tables (Copy + Ln) off the critical path
            dum = small.tile([1, 2], dt.float32)
            nc.vector.memset(dum[:], 1.0)
            dum2 = small.tile([1, 2], dt.float32)
            nc.scalar.copy(dum2[:], dum[:])
            nc.scalar.activation(dum2[:], dum[:], Act.Ln, bias=eps_t[0:1, :], scale=1.0)

            binsA_t = small.tile([128, 1], dt.float32)
            nc.sync.dma_start(binsA_t[:], binsA_d[:])
            binsB_t = small.tile([128, 1], dt.float32)
            nc.sync.dma_start(binsB_t[:], binsB_d[:])
            onesrow_t = small.tile([1, 128], dt.float16)
            nc.sync.dma_start(onesrow_t[:], onesrow_d[:])
            ones_t = small.tile([128, 1], dt.float16)
            nc.sync.dma_start(ones_t[:], ones_d[:])
            ident_t = small.tile([128, 128], dt.float16)
            nc.sync.dma_start(ident_t[:], ident_d[:])
            kt = {}
            for bi in range(2):
                for bo in range(2):
                    kt[bi, bo] = small.tile(
                        [128, 128], dt.float16, tag=f"k{bi}{bo}", name=f"k{bi}{bo}"
                    )
                    nc.sync.dma_start(
                        kt[bi, bo][:],
                        kmat_d[bi * 128 : (bi + 1) * 128, bo * 128 : (bo + 1) * 128],
                    )

            # ---------- stage A (transposed): division image dv [80, 43] ----
            sv_ps = psum1.tile([80, 47], dt.float32, tag="mps")
            nc.tensor.matmul(sv_ps[:], bht[:], xsT[:], start=True, stop=True)
            sv = pre.tile([80, 47], dt.float32)
            nc.scalar.copy(sv[:], sv_ps[:])

            # vertical 5-sum tree over rows (47 -> 43)
            t1a = pre.tile([80, 46], dt.float32)
            nc.vector.tensor_add(t1a[:], sv_ps[:, 0:46], sv[:, 1:47])
            t2a = pre.tile([80, 44], dt.float32)
            nc.vector.tensor_add(t2a[:], t1a[:, 0:44], t1a[:, 2:46])
            s25 = pre.tile([80, ROWS], dt.float32)
            nc.vector.tensor_add(s25[:], t2a[:, 0:43], sv[:, 4:47])

            # smooth = RNE(s25/25)
            tt = pre.tile([80, ROWS], dt.float32)
            nc.vector.tensor_scalar(tt[:], s25[:], INV25, MAGIC, Alu.mult, Alu.add)
            smooth = pre.tile([80, ROWS], dt.float32)
            nc.vector.tensor_scalar(smooth[:], tt[:], MAGIC, None, Alu.subtract)

            # sharp = RNE(clip(2.5 x - 1.25 smooth, 0, 255)); fused round
            sp = pre.tile([80, ROWS], dt.float32)
            nc.vector.scalar_tensor_tensor(
                sp[:], smooth[:], -1.25, xmT[:], Alu.mult, Alu.add
            )
            nc.vector.tensor_scalar(sp[:], sp[:], 0.0, 255.0, Alu.max, Alu.min)
            sharp = pre.tile([80, ROWS], dt.float32)
            nc.vector.tensor_scalar(sharp[:], sp[:], MAGIC, MAGIC, Alu.add, Alu.subtract)

            # division = min(RNE(sharp*255 * recip(smooth+1e-8)), 255)
            denom = pre.tile([80, ROWS], dt.float32)
            nc.vector.tensor_scalar(denom[:], smooth[:], 1e-8, None, Alu.add)
            rr = pre.tile([80, ROWS], dt.float32)
            nc.vector.reciprocal(rr[:], denom[:])
            vv = pre.tile([80, ROWS], dt.float32)
            nc.vector.scalar_tensor_tensor(
                vv[:], sharp[:], 255.0, rr[:], Alu.mult, Alu.mult
            )
            nc.vector.tensor_scalar(tt[:], vv[:], MAGIC, None, Alu.add)
            dv = pre.tile([80, ROWS], dt.float16)
            nc.vector.tensor_scalar(dv[:], tt[:], MAGIC, 255.0, Alu.subtract, Alu.min)

            # flatten to one row, pixel order j = c*43 + r
            dvrow = small.tile([1, NPIX], dt.float16)
            nc.sync.dma_start(dvrow[:], dv[:])

            # ---------- broadcast to 128 partitions (PE) + fp16 copy (ACT) --
            bc16 = big.tile([128, NPIX], dt.float16, tag="bc16")
            boff = 0
            k = 0
            while boff < NPIX:
                bw = min(512, NPIX - boff)
                bcp = psum.tile([128, bw], dt.float32,
                                tag=("g0" if k % 2 == 0 else "g1"), name=f"bc{k}")
                nc.tensor.matmul(
                    bcp[:], onesrow_t[:], dvrow[:, boff : boff + bw],
                    start=True, stop=True,
                )
                nc.scalar.copy(bc16[:, boff : boff + bw], bcp[:])
                boff += bw
                k += 1

            # ---------- one-hot (flat, both halves) ----------
            oh = big.tile([128, NF], dt.float16, tag="oh")
            nc.vector.tensor_scalar(
                oh[:, 0:NPIX], bc16[:], binsA_t[:], None, Alu.is_equal
            )
            nc.vector.tensor_scalar(
                oh[:, NPIX:NF], bc16[:], binsB_t[:], None, Alu.is_equal
            )

            # ---------- vertical 7-sum: flat shift tree (+1) ----------
            # v7[j] = sum oh[j..j+6]; valid where r(j) <= 36
            v1 = big.tile([128, NF], dt.float16, tag="v1")
            nc.vector.tensor_add(v1[:, 0 : NF - 1], oh[:, 0 : NF - 1], oh[:, 1:NF])
            v2 = big.tile([128, NF], dt.float16, tag="v2")
            nc.vector.tensor_add(v2[:, 0 : NF - 3], v1[:, 0 : NF - 3], v1[:, 2 : NF - 1])
            u = big.tile([128, NF], dt.float16, tag="u")
            nc.vector.tensor_add(u[:, 0 : NF - 6], v2[:, 0 : NF - 6], v1[:, 4 : NF - 2])
            v7 = big.tile([128, NF], dt.float16, tag="v7")
            nc.vector.tensor_add(v7[:, 0 : NF - 6], u[:, 0 : NF - 6], oh[:, 6 : NF])

            # ---------- horizontal 7-sum: flat shift tree (+43) ----------
            # t1[j] = v7[j] + v7[j+43]
            t1 = big.tile([128, NF], dt.float16, tag="t1")
            nc.vector.tensor_add(
                t1[:, 0 : NF - 49], v7[:, 0 : NF - 49], v7[:, 43 : NF - 6]
            )
            # DVE computes t2/u2 only for the c < CSPLIT prefix of EACH half
            # (hh c-piece [0:14) reads u2 j<602, u2 reads t2 j<774, t2 reads
            # t1 j<946; within-half shifts of +86/+172 stay inside the half)
            t1h2 = t1[:].rearrange("p (h j) -> p h j", h=2, j=NPIX)
            t2 = big.tile([128, NF], dt.float16, tag="t2")
            t2h2 = t2[:].rearrange("p (h j) -> p h j", h=2, j=NPIX)
            nc.vector.tensor_add(
                t2h2[:, :, 0:776], t1h2[:, :, 0:776], t1h2[:, :, 86:862]
            )
            u2 = big.tile([128, NF], dt.float16, tag="u2")
            u2h2 = u2[:].rearrange("p (h j) -> p h j", h=2, j=NPIX)
            nc.vector.tensor_add(
                u2h2[:, :, 0:604], t2h2[:, :, 0:604], t1h2[:, :, 172:776]
            )

            # views for compacted writes/reads
            v7v = v7[:].rearrange("p (h c r) -> p h c r", h=2, c=80, r=ROWS)
            t1v = t1[:].rearrange("p (h c r) -> p h c r", h=2, c=80, r=ROWS)
            u2v = u2[:].rearrange("p (h c r) -> p h c r", h=2, c=80, r=ROWS)
            hh = big.tile([128, 2 * NP_], dt.float16, tag="hh")
            hv = hh[:].rearrange("p (h c r) -> p h c r", h=2, c=HP, r=PR)

            # hh c in [0, 14): DVE compacted add
            nc.vector.tensor_add(
                hv[:, :, 0:CSPLIT, :],
                u2v[:, :, 0:CSPLIT, 0:PR],
                v7v[:, :, 6 : 6 + CSPLIT, 0:PR],
            )

            # hh c in [14, 74): PE 4-way identity accumulation
            # hh = t1 + t1>>86 + t1>>172 + v7>>258  (shifts in c: +2, +4, +6)
            pe_pieces = []
            for pi in range(5):
                c0 = CSPLIT + 12 * pi
                pw = 12 * PR * 2  # 888 over both halves -> two psum chunks
                pe_pieces.append((pi, c0))
            for pi, c0 in pe_pieces:
                for h in range(2):
                    hp = psum.tile([128, 12 * PR], dt.float32,
                                   tag=("g0" if (pi * 2 + h) % 2 == 0 else "g1"),
                                   name=f"hp{pi}_{h}")
                    nc.tensor.matmul(
                        hp[:], ident_t[:],
                        t1v[:, h, c0 : c0 + 12, 0:PR],
                        start=True, stop=False,
                    )
                    nc.tensor.matmul(
                        hp[:], ident_t[:],
                        t1v[:, h, c0 + 2 : c0 + 14, 0:PR],
                        start=False, stop=False,
                    )
                    nc.tensor.matmul(
                        hp[:], ident_t[:],
                        t1v[:, h, c0 + 4 : c0 + 16, 0:PR],
                        start=False, stop=False,
                    )
                    nc.tensor.matmul(
                        hp[:], ident_t[:],
                        v7v[:, h, c0 + 6 : c0 + 18, 0:PR],
                        start=False, stop=True,
                    )
                    nc.scalar.copy(hv[:, h, c0 : c0 + 12, :], hp[:])

            # ---------- stage C: G -> Ln -> h*log -> (-1/49) reduce ---------
            ent_row = small.tile([1, NP_], dt.float32)
            for ci, (off, cw) in enumerate(cchunks):
                h0c = hh[:, off : off + cw]
                h1c = hh[:, NP_ + off : NP_ + off + cw]
                g0 = psum.tile([128, cw], dt.float32, tag="g0", name=f"g0_{ci}")
                nc.tensor.matmul(g0[:], kt[0, 0][:], h0c, start=True, stop=False)
                nc.tensor.matmul(g0[:], kt[1, 0][:], h1c, start=False, stop=True)
                g1 = psum.tile([128, cw], dt.float32, tag="g1", name=f"g1_{ci}")
                nc.tensor.matmul(g1[:], kt[0, 1][:], h0c, start=True, stop=False)
                nc.tensor.matmul(g1[:], kt[1, 1][:], h1c, start=False, stop=True)
                lp0 = scratch.tile([128, cw], dt.float16, tag="lp0", name=f"lp0_{ci}", bufs=3)
                nc.scalar.activation(
                    lp0[:], g0[:], Act.Ln, bias=eps_t[:], scale=LN_SCALE
                )
                lp1 = scratch.tile([128, cw], dt.float16, tag="lp1", name=f"lp1_{ci}", bufs=3)
                nc.scalar.activation(
                    lp1[:], g1[:], Act.Ln, bias=eps_t[:], scale=LN_SCALE
                )
                m0 = scratch.tile([128, cw], dt.float16, tag="m0", name=f"m0_{ci}", bufs=3)
                nc.vector.tensor_mul(m0[:], h0c, lp0[:])
                m1 = scratch.tile([128, cw], dt.float16, tag="m1", name=f"m1_{ci}", bufs=3)
                nc.vector.tensor_mul(m1[:], h1c, lp1[:])
                psc = scratch.tile([128, cw], dt.float16, tag="psc", name=f"psc_{ci}", bufs=3)
                nc.vector.tensor_add(psc[:], m0[:], m1[:])
                e_ps = psum1.tile([1, cw], dt.float32, tag="mps", name=f"e_{ci}")
                nc.tensor.matmul(e_ps[:], ones_t[:], psc[:], start=True, stop=True)
                nc.scalar.copy(ent_row[:, off : off + cw], e_ps[:])
            nc.sync.dma_start(ent_d[:], ent_row[:])

    nc.compile()
    return nc


def _get_compiled():
    global _COMPILED
    if _COMPILED is None:
        _COMPILED = (_build_nc(), _host_constants())
    return _COMPILED


def _run(x, trace=False, **kw):
    """x: (2,2,1,80,80) float32. Returns BassKernelResults."""
    nc, consts = _get_compiled()
    xi = np.ascontiguousarray(np.asarray(x, f32).reshape(4, 80, 80))
    in_maps = []
    for core in range(8):
        b, half = core // 2, core % 2
        r0 = half * PR
        strip = np.zeros((47, 80), f32)
        lo, hi = r0 - 2, r0 + 45
        slo, shi = max(lo, 0), min(hi, 80)
        strip[slo - lo : shi - lo] = xi[b, slo:shi]
        xsT = np.zeros((84, 47), f32)
        xsT[2:82, :] = strip.T
        m = dict(consts)
        m["xsT"] = xsT
        m["xmT"] = np.ascontiguousarray(f32(2.5) * strip[2 : 2 + ROWS].T)
        in_maps.append(m)
    res = run_bass_kernel_spmd(nc, in_maps, list(range(8)), trace=trace, **kw)
    return res


def kernel(x):
    res = _run(x)
    out = np.zeros((4, 80, 80), f32)
    pad = R // 2
    for core in range(8):
        b, half = core // 2, core % 2
        r0 = half * PR
        ent = np.asarray(res.results[core]["ent"], f32).reshape(HP, PR).T
        out[b, pad + r0 : pad + r0 + PR, pad : pad + HP] = ent
    return out.reshape(2, 2, 80, 80)
